# revision 18
# baseline (speedup 1.0000x reference)
"""AttnBlock Trainium2 Bass kernel.

Data-parallel over batch across 8 NeuronCores (4 batch elements each, full
weights on every core). Everything on-chip is feature-major ([feat, token]),
so the pipeline needs no transposes anywhere.

The kernel is paced by the Scalar engine: softmax exp is 16.8M elements per
core and ACT runs 1 elem/lane/cycle @1.2GHz => ~147us floor (128 ACT ops of
[128, 1024]). Everything else is sized to hide under that:

  PE work uses fp8e4m3 + DoubleRow perf mode (2 weights/cell => contraction
  256 per pass) for the QKV/out projections and the ctx matmul, which cuts
  PE cycles/batch to ~13us warm -- the PE fits under the ACT pace even when
  the HAM clock-gate has it at 1.2GHz, so throttling can't stall the stream.
  Scores stay bf16 (d_k=64 contraction can't DoubleRow without extra
  copies) but the head pair is row-tiled (tile_position (0,0)/(64,0)) and
  issued back-to-back so both heads stream concurrently.

  x[b]  -> xq [128, 2cc, N] fp8 (host-packed; cc = C/128 contraction pair)
  QK    -> DR matmul, psum -> bf16 Qst/Kst [128, N] (head pair 64+64 rows),
           bias folded into the DVE copy
  V     -> DR matmul -> V [N, 4, 68pad] fp8, [token, dim]; bias + ones col
           (softmax denominator trick) folded into the DVE copy
  scores-> sp [128, 2, 512] psum: both heads x one i-half; two sp tiles
           ping-pong so next scores overlap the current exp
  P     -> ONE ACT exp per (jc, ic) -> fp8 into jc-pair tiles
           [128, 2jq, 2hl, 512] laid out for the DoubleRow moving operand
  ctx   -> DR matmul over jc pairs -> psum [65, N]/head; row 64 = Z
  norm  -> DVE copies free the c-slot, GPSIMD partition_broadcast of Z,
           DVE reciprocal + multiply -> cnb [128, 2pack, N] fp8 (the
           DoubleRow moving layout for the out projection)
  out   -> DR matmul; bias + fp32 residual fused in the DVE psum->sbuf pass

PSUM (8 banks, exactly full): sp0, sp1 [128,1024] (scores ping-pong, also
time-shared by the projection psums, which are emitted as <=1us fillers one
per jc inside the attention loops so the in-order PE queue never stalls);
c0, c1 [65,1024] ctx accumulators, released right after each pack by the
DVE copies.

Filler schedule per batch b (one before each jc's scores):
  pack0: V(b+1) pair0/1, Q1(b) half0/1, K1(b) half0/1, outproj(b-1) co0
  pack1: outproj(b-1) co1, V(b+1) pair2/3, Q0(b+1) half0/1, K0(b+1) half0/1
so next-batch Q/K are ready exactly at the batch boundary and the ACT
stream never waits on projections.

Final rel err vs the fp32 reference: ~2e-3 (fp8 weights/activations on the
attention path; tolerance is 2e-2).
"""

import numpy as np
import ml_dtypes

N_HEADS = 4
D_K = 64
SCALE = D_K ** (-0.5)
B, C, H, W = 32, 256, 32, 32
N = H * W           # 1024 tokens
NCORES = 8
BPC = B // NCORES   # 4 batch elements per core

_CACHE = {}


def _build():
    import concourse.bacc as bacc
    import concourse.mybir as mybir
    from concourse.tile import TileContext

    dt = mybir.dt
    f32 = dt.float32
    bf16 = dt.bfloat16
    f8 = dt.float8e4
    DR = mybir.MatmulPerfMode.DoubleRow
    EXP = mybir.ActivationFunctionType.Exp
    ADD = mybir.AluOpType.add
    MULT = mybir.AluOpType.mult

    nc = bacc.Bacc()
    x = nc.dram_tensor("x", [BPC, C, N], f32, kind="ExternalInput")
    xq = nc.dram_tensor("xq", [BPC, 128, 2, N], f8, kind="ExternalInput")
    wqkdr = nc.dram_tensor("wqkdr", [128, 2, 512], f8, kind="ExternalInput")
    bqk = nc.dram_tensor("bqk", [128, 4], f32, kind="ExternalInput")
    wvdr = nc.dram_tensor("wvdr", [128, 2, 272], f8, kind="ExternalInput")
    wvb2 = nc.dram_tensor("wvb2", [128, 2, 272], f32, kind="ExternalInput")
    wodr = nc.dram_tensor("wodr", [128, 2, 256], f8, kind="ExternalInput")
    ob = nc.dram_tensor("ob", [128, 2], f32, kind="ExternalInput")
    out = nc.dram_tensor("out", [BPC, C, N], f32, kind="ExternalOutput")

    with TileContext(nc) as tc:
        with (
            tc.tile_pool(name="consts", bufs=1) as consts,
            tc.tile_pool(name="xp", bufs=4) as xp,
            tc.tile_pool(name="qkp", bufs=4) as qkp,
            tc.tile_pool(name="vp", bufs=2) as vp,
            tc.tile_pool(name="pp", bufs=2) as pp,
            tc.tile_pool(name="miscp", bufs=4) as miscp,
            tc.tile_pool(name="outp", bufs=4) as outp,
            tc.tile_pool(name="psum", bufs=1, space="PSUM") as psum,
        ):
            # ---- load constants once (already fp8 host-side) ----
            wqk_sb = consts.tile([128, 2, 512], f8, name="wqk_sb")
            wv_sb = consts.tile([128, 2, 272], f8, name="wv_sb")
            wo_sb = consts.tile([128, 2, 256], f8, name="wo_sb")
            bqk_sb = consts.tile([128, 4], f32, name="bqk_sb")
            wvb2_sb = consts.tile([128, 2, 272], f32, name="wvb2_sb")
            ob_sb = consts.tile([128, 2], f32, name="ob_sb")
            nc.sync.dma_start(out=wqk_sb[:], in_=wqkdr[:])
            nc.sync.dma_start(out=wv_sb[:], in_=wvdr[:])
            nc.sync.dma_start(out=wo_sb[:], in_=wodr[:])
            nc.sync.dma_start(out=bqk_sb[:], in_=bqk[:])
            nc.sync.dma_start(out=wvb2_sb[:], in_=wvb2[:])
            nc.sync.dma_start(out=ob_sb[:], in_=ob[:])
            warmup = consts.tile([1, 4], f32, name="warmup")
            nc.scalar.activation(warmup[:], bqk_sb[0:1, 0:4], EXP)

            xcs, xqs, qks, vss, osbs, cnbs = {}, {}, {}, {}, {}, {}

            def emit_x_load(b):
                xc = [xp.tile([128, N], f32, name=f"xc{cc}", tag=f"xc{cc}")
                      for cc in range(2)]
                for cc in range(2):
                    nc.sync.dma_start(out=xc[cc][:], in_=x[b, cc * 128:(cc + 1) * 128, :])
                xq_sb = xp.tile([128, 2, N], f8, name="xq_sb", tag="xq")
                nc.sync.dma_start(out=xq_sb[:], in_=xq[b])
                xcs[b] = xc
                xqs[b] = xq_sb

            def emit_qk_half(b, p, qk, fc):
                # one fc-half of one Q/K projection tile (1 DoubleRow MM);
                # psum borrows the sp slots
                if b not in qks:
                    qks[b] = [[None, None], [None, None]]
                qkps = psum.tile([128, 512], f32, name="qkps", tag=f"sp{qk}")
                col0 = p * 256 + qk * 128
                fs = slice(fc * 512, (fc + 1) * 512)
                nc.tensor.matmul(
                    qkps[:],
                    wqk_sb[:, :, col0:col0 + 128],
                    xqs[b][:, :, fs],
                    start=True, stop=True, perf_mode=DR,
                )
                if fc == 0:
                    qks[b][p][qk] = qkp.tile([128, N], bf16, name=f"qk{p}{qk}")
                nc.vector.tensor_scalar(
                    qks[b][p][qk][:, fs], qkps[:],
                    bqk_sb[:, 2 * p + qk:2 * p + qk + 1],
                    None, ADD,
                )

            def emit_v_pair(b, pr):
                # two 128-token V chunks (1 DR MM each) + one DVE op
                if b not in vss:
                    vss[b] = vp.tile([128, 8, 272], f8, name="v_sb", tag="v")
                vps = psum.tile([128, 2, 512], f32, name="vps", tag=f"sp{pr % 2}")
                for k in range(2):
                    jt = 2 * pr + k
                    js = slice(jt * 128, (jt + 1) * 128)
                    nc.tensor.matmul(
                        vps[:, k, 0:272],
                        xqs[b][:, :, js], wv_sb[:],
                        start=True, stop=True, perf_mode=DR,
                    )
                nc.vector.scalar_tensor_tensor(
                    vss[b][:, 2 * pr:2 * pr + 2, :], vps[:, :, 0:272],
                    1.0, wvb2_sb[:], MULT, ADD,
                )

            def emit_outproj_half(b, co, fc):
                if (b, co) not in osbs:
                    osbs[(b, co)] = outp.tile([128, N], f32, name="osb")
                osb = osbs[(b, co)]
                fs = slice(fc * 512, (fc + 1) * 512)
                ops = psum.tile([128, 512], f32, name="ops", tag=f"sp{co}")
                nc.tensor.matmul(
                    ops[:],
                    wo_sb[:, :, co * 128:(co + 1) * 128],
                    cnbs[b][:, :, fs],
                    start=True, stop=True, perf_mode=DR,
                )
                nc.vector.scalar_tensor_tensor(
                    osb[:, fs], ops[:], ob_sb[:, co:co + 1], xcs[b][co][:, fs],
                    ADD, ADD,
                )
                if fc == 1:
                    nc.sync.dma_start(
                        out=out[b, co * 128:(co + 1) * 128, :], in_=osb[:]
                    )

            def emit_pack(b, p, fillers):
                qst, kst = qks[b][p][0], qks[b][p][1]
                v_sb = vss[b]
                ctxps = [
                    psum.tile([65, N], f32, name=f"ctx{hl}", tag=f"c{hl}")
                    for hl in range(2)
                ]
                for jcp in range(4):
                    ppq = [pp.tile([128, 2, 2, 512], f8, name="ppq",
                                   tag=f"pp{ic}") for ic in range(2)]
                    for jq in range(2):
                        jc = 2 * jcp + jq
                        if fillers[jc] is not None:
                            fillers[jc]()
                        js = slice(jc * 128, (jc + 1) * 128)
                        for ic in range(2):
                            isl = slice(ic * 512, (ic + 1) * 512)
                            sp = psum.tile([128, 2, 512], f32, name="sp",
                                           tag=f"sp{ic}")
                            for hl in range(2):
                                hs = slice(hl * 64, (hl + 1) * 64)
                                nc.tensor.matmul(
                                    sp[:, hl, :],
                                    kst[hs, js],
                                    qst[hs, isl],
                                    start=True, stop=True,
                                    tile_position=(hl * 64, 0),
                                )
                            nc.scalar.activation(
                                ppq[ic][:, jq, :, :], sp[:], EXP, scale=SCALE
                            )
                    for ic in range(2):
                        isl = slice(ic * 512, (ic + 1) * 512)
                        for hl in range(2):
                            h = 2 * p + hl
                            nc.tensor.matmul(
                                ctxps[hl][:, isl],
                                v_sb[:, 2 * jcp:2 * jcp + 2, h * 68:h * 68 + 65],
                                ppq[ic][:, :, hl, :],
                                start=(jcp == 0), stop=(jcp == 3),
                                perf_mode=DR,
                            )
                # normalize: copies free the ctx psum slot; broadcast must
                # source partition 0, so the Z row is copied down first
                if p == 0:
                    cnbs[b] = miscp.tile([128, 2, N], f8, name="cnb",
                                         tag="cn", bufs=3)
                for hl in range(2):
                    cu = miscp.tile([64, N], f32, name="cu", tag="cu", bufs=3)
                    nc.vector.tensor_copy(cu[:], ctxps[hl][0:64, :])
                    z_sb = miscp.tile([1, N], f32, name="z_sb", tag="z", bufs=3)
                    nc.vector.tensor_copy(z_sb[:], ctxps[hl][64:65, :])
                    zb = miscp.tile([64, N], f32, name="zb", tag="zb", bufs=2)
                    nc.gpsimd.partition_broadcast(zb[:], z_sb[0:1, :])
                    rzb = miscp.tile([64, N], f32, name="rzb", tag="rzb", bufs=2)
                    nc.vector.reciprocal_approx_fast(rzb[:], zb[:])
                    nc.vector.tensor_tensor(
                        cnbs[b][hl * 64:(hl + 1) * 64, p, :],
                        cu[:],
                        rzb[:],
                        MULT,
                    )

            # prologue: batch 0's pack-0 Q/K + all of V, plus x loads
            emit_x_load(0)
            for qk in range(2):
                for fc in range(2):
                    emit_qk_half(0, 0, qk, fc)
            for pr in range(4):
                emit_v_pair(0, pr)
            emit_x_load(1)

            def sched(b):
                """filler lists for pack0 / pack1 of batch b"""
                nxt = b + 1 if b + 1 < BPC else None
                prv = b - 1 if b >= 1 else None
                f0 = [
                    (lambda: emit_v_pair(nxt, 0)) if nxt is not None else None,
                    (lambda: emit_v_pair(nxt, 1)) if nxt is not None else None,
                    (lambda: emit_qk_half(b, 1, 0, 0)),
                    (lambda: emit_qk_half(b, 1, 0, 1)),
                    (lambda: emit_qk_half(b, 1, 1, 0)),
                    (lambda: emit_qk_half(b, 1, 1, 1)),
                    (lambda: emit_outproj_half(prv, 0, 0)) if prv is not None else None,
                    (lambda: emit_outproj_half(prv, 0, 1)) if prv is not None else None,
                ]
                f1 = [
                    (lambda: emit_outproj_half(prv, 1, 0)) if prv is not None else None,
                    (lambda: emit_outproj_half(prv, 1, 1)) if prv is not None else None,
                    (lambda: emit_v_pair(nxt, 2)) if nxt is not None else None,
                    (lambda: emit_v_pair(nxt, 3)) if nxt is not None else None,
                    (lambda: emit_qk_half(nxt, 0, 0, 0)) if nxt is not None else None,
                    (lambda: emit_qk_half(nxt, 0, 0, 1)) if nxt is not None else None,
                    (lambda: emit_qk_half(nxt, 0, 1, 0)) if nxt is not None else None,
                    (lambda: emit_qk_half(nxt, 0, 1, 1)) if nxt is not None else None,
                ]
                return f0, f1

            for b in range(BPC):
                f0, f1 = sched(b)
                emit_pack(b, 0, f0)
                emit_pack(b, 1, f1)
                if b + 2 < BPC:
                    emit_x_load(b + 2)
            # tail: last batch's out projection
            for co in range(2):
                for fc in range(2):
                    emit_outproj_half(BPC - 1, co, fc)

    nc.compile()
    return nc


def _prep_weights(proj_w, proj_b, out_w, out_b):
    f8 = ml_dtypes.float8_e4m3
    qk_cols = []
    for p in range(2):
        for qk in range(2):
            for hl in range(2):
                h = 2 * p + hl
                base = h * 192 + qk * 64
                qk_cols.extend(range(base, base + 64))
    wqk = proj_w[qk_cols, :].T                                # [256, 512]
    wqkdr = np.ascontiguousarray(
        wqk.reshape(2, 128, 512).transpose(1, 0, 2)           # [128, 2, 512]
    ).astype(f8)
    bqk = np.ascontiguousarray(proj_b[qk_cols].reshape(4, 128).T)

    # V weights: per head 68-wide padded block, ones col at +64 (bias-only)
    wv = np.zeros((C, 272), dtype=np.float32)
    wvb1 = np.zeros((1, 272), dtype=np.float32)
    for h in range(N_HEADS):
        rows = range(h * 192 + 128, h * 192 + 192)
        wv[:, h * 68:h * 68 + 64] = proj_w[rows, :].T
        wvb1[0, h * 68:h * 68 + 64] = proj_b[rows]
        wvb1[0, h * 68 + 64] = 1.0
    wvdr = np.ascontiguousarray(
        wv.reshape(2, 128, 272).transpose(1, 0, 2)            # [128, 2, 272]
    ).astype(f8)
    wvb2 = np.ascontiguousarray(
        np.broadcast_to(wvb1[None, :, :], (128, 2, 272))      # [128, 2, 272]
    )

    wo = out_w.T                                              # [256, 256]
    wodr = np.ascontiguousarray(
        wo.reshape(2, 128, 256).transpose(1, 0, 2)            # [128, 2, 256]
    ).astype(f8)
    ob = np.ascontiguousarray(out_b.reshape(2, 128).T)        # [128, 2]
    return dict(wqkdr=wqkdr, bqk=bqk, wvdr=wvdr, wvb2=wvb2, wodr=wodr, ob=ob)


def kernel(x, proj_w, proj_b, out_w, out_b, _trace=False):
    from concourse.bass_utils import run_bass_kernel_spmd

    x = np.asarray(x, dtype=np.float32)
    proj_w = np.asarray(proj_w, dtype=np.float32)
    proj_b = np.asarray(proj_b, dtype=np.float32)
    out_w = np.asarray(out_w, dtype=np.float32)
    out_b = np.asarray(out_b, dtype=np.float32)

    if "nc" not in _CACHE:
        _CACHE["nc"] = _build()
    nc = _CACHE["nc"]

    w = _prep_weights(proj_w, proj_b, out_w, out_b)
    xs = np.ascontiguousarray(x.reshape(B, C, N))
    # xq[b, p, c, n] = x[b, c*128 + p, n], fp8
    xqh = np.ascontiguousarray(
        xs.reshape(B, 2, 128, N).transpose(0, 2, 1, 3)
    ).astype(ml_dtypes.float8_e4m3)
    in_maps = [
        dict(w, x=np.ascontiguousarray(xs[i * BPC:(i + 1) * BPC]),
             xq=np.ascontiguousarray(xqh[i * BPC:(i + 1) * BPC]))
        for i in range(NCORES)
    ]
    res = run_bass_kernel_spmd(nc, in_maps, core_ids=list(range(NCORES)), trace=_trace)
    out = np.concatenate([r["out"] for r in res.results], axis=0)
    out = out.reshape(B, C, H, W)
    if _trace:
        _CACHE["last_result"] = res
    return out


# revision 19
# speedup vs baseline: 1.3554x; 1.3554x over previous
"""AttnBlock Trainium2 Bass kernel.

Data-parallel over batch across 8 NeuronCores (4 batch elements each, full
weights on every core). Everything on-chip is feature-major ([feat, token]),
so the pipeline needs no transposes anywhere.

The kernel is paced by the Scalar engine: softmax exp is 16.8M elements per
core and ACT runs 1 elem/lane/cycle @1.2GHz => ~147us floor (128 ACT ops of
[128, 1024]). The structure keeps every other engine hidden under that:

  x[b]                -> X [C=256, N=1024] (the input's natural layout)
  QK proj (PE bf16)   -> Qst/Kst [128, N], head pair stacked 64+64 rows;
                         bias folded into the DVE psum->sbuf copy
  V proj              -> V [N, 4, 65] [token, dim]; bias + ones column
                         (softmax denominator trick) folded into the copy
  scores              -> sp [128, 2, 512] psum: both heads x one i-half;
                         the head pair is row-tiled (tile_position (0,0) /
                         (64,0)) and issued back-to-back so both heads
                         stream through the PE array concurrently; two sp
                         tiles ping-pong so next scores overlap current exp
  P = exp(scale*s)    -> ONE ACT op per (jc, ic) [128, 1024] psum -> bf16
  ctx = [V|1]^T P     -> psum [65, N] per head; row 64 accumulates Z.
                         ctx matmuls are emitted TWO periods late so the
                         in-order PE queue never parks on the ctx-slot WAR
                         (released by the copy below) while ACT starves.
  normalize           -> ONE DVE copy [65, N] frees the ctx psum slot;
                         Z row is then copied out of SBUF (partition 0),
                         GPSIMD partition_broadcast, DVE recip + multiply
  out proj            -> OUT^T [C, N]; bias + fp32 residual fused in the
                         DVE psum->sbuf pass

PSUM (8 banks, exactly full): sp0, sp1 [128,1024] scores ping-pong, also
time-shared by the projection psums, which are emitted as <=1.1us fillers
one per jc inside the attention loops (the PE slack under the ACT pace);
c0, c1 [65,1024] ctx accumulators, released 1.2us after each pack.

Filler schedule per batch b (one before each jc's scores):
  pack0: V(b+1) pair0/1, Q1(b) half0/1, K1(b) half0/1, outproj(b-1) co0
  pack1: outproj(b-1) co1, V(b+1) pair2/3, Q0(b+1) half0/1, K0(b+1) half0/1
so next-batch Q/K are ready exactly at the batch boundary and the ACT
stream never waits on projections.

Matmul operands are bf16 (converted host-side; fp32 PSUM accumulation).
"""

import numpy as np
import ml_dtypes

N_HEADS = 4
D_K = 64
SCALE = D_K ** (-0.5)
B, C, H, W = 32, 256, 32, 32
N = H * W           # 1024 tokens
NCORES = 8
BPC = B // NCORES   # 4 batch elements per core

_CACHE = {}


def _build():
    import concourse.bacc as bacc
    import concourse.mybir as mybir
    from concourse.tile import TileContext

    dt = mybir.dt
    f32 = dt.float32
    bf16 = dt.bfloat16
    EXP = mybir.ActivationFunctionType.Exp
    ADD = mybir.AluOpType.add
    MULT = mybir.AluOpType.mult

    nc = bacc.Bacc()
    x = nc.dram_tensor("x", [BPC, C, N], f32, kind="ExternalInput")
    xbf = nc.dram_tensor("xbf", [BPC, C, N], bf16, kind="ExternalInput")
    wqk = nc.dram_tensor("wqk", [C, 512], bf16, kind="ExternalInput")
    bqk = nc.dram_tensor("bqk", [128, 4], f32, kind="ExternalInput")
    wv = nc.dram_tensor("wv", [C, 260], bf16, kind="ExternalInput")
    wvb2 = nc.dram_tensor("wvb2", [128, 520], f32, kind="ExternalInput")
    wo = nc.dram_tensor("wo", [C, C], bf16, kind="ExternalInput")
    ob = nc.dram_tensor("ob", [128, 2], f32, kind="ExternalInput")
    out = nc.dram_tensor("out", [BPC, C, N], f32, kind="ExternalOutput")

    with TileContext(nc) as tc:
        with (
            tc.tile_pool(name="consts", bufs=1) as consts,
            tc.tile_pool(name="xp", bufs=4) as xp,
            tc.tile_pool(name="qkp", bufs=4) as qkp,
            tc.tile_pool(name="vp", bufs=2) as vp,
            tc.tile_pool(name="pp", bufs=6) as pp,
            tc.tile_pool(name="miscp", bufs=4) as miscp,
            tc.tile_pool(name="outp", bufs=4) as outp,
            tc.tile_pool(name="psum", bufs=1, space="PSUM") as psum,
        ):
            # ---- load constants once (already bf16 host-side) ----
            wqk_sb = [consts.tile([128, 512], bf16, name=f"wqk{cc}") for cc in range(2)]
            wv_sb = [consts.tile([128, 260], bf16, name=f"wv{cc}") for cc in range(2)]
            wo_sb = [consts.tile([128, 256], bf16, name=f"wo{cc}") for cc in range(2)]
            bqk_sb = consts.tile([128, 4], f32, name="bqk_sb")
            wvb2_sb = consts.tile([128, 520], f32, name="wvb2_sb")
            ob_sb = consts.tile([128, 2], f32, name="ob_sb")
            for cc in range(2):
                nc.sync.dma_start(out=wqk_sb[cc][:], in_=wqk[cc * 128:(cc + 1) * 128, :])
                nc.sync.dma_start(out=wv_sb[cc][:], in_=wv[cc * 128:(cc + 1) * 128, :])
                nc.sync.dma_start(out=wo_sb[cc][:], in_=wo[cc * 128:(cc + 1) * 128, :])
            nc.sync.dma_start(out=bqk_sb[:], in_=bqk[:])
            nc.sync.dma_start(out=wvb2_sb[:], in_=wvb2[:])
            nc.sync.dma_start(out=ob_sb[:], in_=ob[:])
            warmup = consts.tile([1, 4], f32, name="warmup")
            nc.scalar.activation(warmup[:], bqk_sb[0:1, 0:4], EXP)

            xcs, xcrs, qks, vss, osbs, cns = {}, {}, {}, {}, {}, {}

            def emit_x_load(b):
                xc = [xp.tile([128, N], f32, name=f"xc{cc}", tag=f"xc{cc}")
                      for cc in range(2)]
                xcr = [xp.tile([128, N], bf16, name=f"xcr{cc}", tag=f"xcr{cc}")
                       for cc in range(2)]
                for cc in range(2):
                    nc.sync.dma_start(out=xc[cc][:], in_=x[b, cc * 128:(cc + 1) * 128, :])
                    nc.sync.dma_start(out=xcr[cc][:], in_=xbf[b, cc * 128:(cc + 1) * 128, :])
                xcs[b] = xc
                xcrs[b] = xcr

            def emit_qk_half(b, p, qk, fc):
                # one fc-half of one Q/K projection tile; psum borrows the
                # sp slots so it never waits on the normalize chain
                if b not in qks:
                    qks[b] = [[None, None], [None, None]]
                xcr = xcrs[b]
                qkps = psum.tile([128, 512], f32, name="qkps", tag=f"sp{qk}")
                col0 = p * 256 + qk * 128
                fs = slice(fc * 512, (fc + 1) * 512)
                for cc in range(2):
                    nc.tensor.matmul(
                        qkps[:],
                        wqk_sb[cc][:, col0:col0 + 128],
                        xcr[cc][:, fs],
                        start=(cc == 0), stop=(cc == 1),
                    )
                if fc == 0:
                    qks[b][p][qk] = qkp.tile([128, N], bf16, name=f"qk{p}{qk}")
                nc.vector.tensor_scalar(
                    qks[b][p][qk][:, fs], qkps[:],
                    bqk_sb[:, 2 * p + qk:2 * p + qk + 1],
                    None, ADD,
                )

            def emit_v_pair(b, pr):
                # two 128-token V chunks through one psum tile + one DVE op
                if b not in vss:
                    vss[b] = vp.tile([128, 8, 260], bf16, name="v_sb", tag="v")
                xcr = xcrs[b]
                vps = psum.tile([128, 2, 512], f32, name="vps", tag=f"sp{pr % 2}")
                for k in range(2):
                    jt = 2 * pr + k
                    js = slice(jt * 128, (jt + 1) * 128)
                    for cc in range(2):
                        nc.tensor.matmul(
                            vps[:, k, 0:260],
                            xcr[cc][:, js], wv_sb[cc][:],
                            start=(cc == 0), stop=(cc == 1),
                        )
                nc.vector.scalar_tensor_tensor(
                    vss[b][:, 2 * pr:2 * pr + 2, :], vps[:, :, 0:260],
                    1.0, wvb2_sb[:], MULT, ADD,
                )

            def emit_outproj_half(b, co, fc):
                if (b, co) not in osbs:
                    osbs[(b, co)] = outp.tile([128, N], f32, name="osb")
                osb = osbs[(b, co)]
                ctxn = cns[b]
                fs = slice(fc * 512, (fc + 1) * 512)
                ops = psum.tile([128, 512], f32, name="ops", tag=f"sp{co}")
                for kc in range(2):
                    nc.tensor.matmul(
                        ops[:],
                        wo_sb[kc][:, co * 128:(co + 1) * 128],
                        ctxn[kc][:, fs],
                        start=(kc == 0), stop=(kc == 1),
                    )
                nc.vector.scalar_tensor_tensor(
                    osb[:, fs], ops[:], ob_sb[:, co:co + 1], xcs[b][co][:, fs],
                    ADD, ADD,
                )
                if fc == 1:
                    nc.sync.dma_start(
                        out=out[b, co * 128:(co + 1) * 128, :], in_=osb[:]
                    )

            def emit_pack(b, p, fillers):
                qst, kst = qks[b][p][0], qks[b][p][1]
                v_sb = vss[b]
                ctxps = [
                    psum.tile([65, N], f32, name=f"ctx{hl}", tag=f"c{hl}")
                    for hl in range(2)
                ]
                pending = []  # ctx matmuls delayed two periods

                def emit_ctx(jc, ic, pt):
                    isl = slice(ic * 512, (ic + 1) * 512)
                    for hl in range(2):
                        h = 2 * p + hl
                        nc.tensor.matmul(
                            ctxps[hl][:, isl],
                            v_sb[:, jc, h * 65:(h + 1) * 65],
                            pt[:, hl, :],
                            start=(jc == 0), stop=(jc == 7),
                        )

                for jc in range(8):
                    if fillers[jc] is not None:
                        fillers[jc]()
                    js = slice(jc * 128, (jc + 1) * 128)
                    for ic in range(2):
                        isl = slice(ic * 512, (ic + 1) * 512)
                        sp = psum.tile([128, 2, 512], f32, name="sp",
                                       tag=f"sp{ic}")
                        for hl in range(2):
                            hs = slice(hl * 64, (hl + 1) * 64)
                            nc.tensor.matmul(
                                sp[:, hl, :],
                                kst[hs, js],
                                qst[hs, isl],
                                start=True, stop=True,
                                tile_position=(hl * 64, 0),
                            )
                        pt = pp.tile([128, 2, 512], bf16, name="pt", tag="pt")
                        nc.scalar.activation(pt[:], sp[:], EXP, scale=SCALE)
                        pending.append((jc, ic, pt))
                        if len(pending) > 2:
                            emit_ctx(*pending.pop(0))
                for args in pending:
                    emit_ctx(*args)
                # normalize: single copy frees the ctx psum slot; Z row is
                # re-copied from SBUF down to partition 0 (gpsimd broadcast
                # can only source partition 0), everything else off-path
                cn = miscp.tile([128, N], bf16, name=f"ctxn{p}", tag="cn")
                for hl in range(2):
                    cu = miscp.tile([65, N], f32, name="cu", tag="cu", bufs=3)
                    nc.vector.tensor_copy(cu[:], ctxps[hl][:])
                    z_sb = miscp.tile([1, N], f32, name="z_sb", tag="z", bufs=3)
                    nc.vector.tensor_copy(z_sb[:], cu[64:65, :])
                    zb = miscp.tile([64, N], f32, name="zb", tag="zb", bufs=2)
                    nc.gpsimd.partition_broadcast(zb[:], z_sb[0:1, :])
                    rzb = miscp.tile([64, N], f32, name="rzb", tag="rzb", bufs=2)
                    nc.vector.reciprocal_approx_fast(rzb[:], zb[:])
                    nc.vector.tensor_tensor(
                        cn[hl * 64:(hl + 1) * 64, :],
                        cu[0:64, :],
                        rzb[:],
                        MULT,
                    )
                return cn

            # prologue: batch 0's pack-0 Q/K + all of V, plus x loads
            emit_x_load(0)
            for qk in range(2):
                for fc in range(2):
                    emit_qk_half(0, 0, qk, fc)
            for pr in range(4):
                emit_v_pair(0, pr)
            emit_x_load(1)

            def sched(b):
                """filler lists for pack0 / pack1 of batch b"""
                nxt = b + 1 if b + 1 < BPC else None
                prv = b - 1 if b >= 1 else None
                f0 = [
                    (lambda: emit_v_pair(nxt, 0)) if nxt is not None else None,
                    (lambda: emit_v_pair(nxt, 1)) if nxt is not None else None,
                    (lambda: emit_qk_half(b, 1, 0, 0)),
                    (lambda: emit_qk_half(b, 1, 0, 1)),
                    (lambda: emit_qk_half(b, 1, 1, 0)),
                    (lambda: emit_qk_half(b, 1, 1, 1)),
                    (lambda: emit_outproj_half(prv, 0, 0)) if prv is not None else None,
                    (lambda: emit_outproj_half(prv, 0, 1)) if prv is not None else None,
                ]
                f1 = [
                    (lambda: emit_outproj_half(prv, 1, 0)) if prv is not None else None,
                    (lambda: emit_outproj_half(prv, 1, 1)) if prv is not None else None,
                    (lambda: emit_v_pair(nxt, 2)) if nxt is not None else None,
                    (lambda: emit_v_pair(nxt, 3)) if nxt is not None else None,
                    (lambda: emit_qk_half(nxt, 0, 0, 0)) if nxt is not None else None,
                    (lambda: emit_qk_half(nxt, 0, 0, 1)) if nxt is not None else None,
                    (lambda: emit_qk_half(nxt, 0, 1, 0)) if nxt is not None else None,
                    (lambda: emit_qk_half(nxt, 0, 1, 1)) if nxt is not None else None,
                ]
                return f0, f1

            for b in range(BPC):
                f0, f1 = sched(b)
                cn0 = emit_pack(b, 0, f0)
                cn1 = emit_pack(b, 1, f1)
                cns[b] = [cn0, cn1]
                if b + 2 < BPC:
                    emit_x_load(b + 2)
            # tail: last batch's out projection
            for co in range(2):
                for fc in range(2):
                    emit_outproj_half(BPC - 1, co, fc)

    nc.compile()
    return nc


def _prep_weights(proj_w, proj_b, out_w, out_b):
    qk_cols = []
    for p in range(2):
        for qk in range(2):
            for hl in range(2):
                h = 2 * p + hl
                base = h * 192 + qk * 64
                qk_cols.extend(range(base, base + 64))
    wqk = np.ascontiguousarray(proj_w[qk_cols, :].T).astype(ml_dtypes.bfloat16)
    bqk = np.ascontiguousarray(proj_b[qk_cols].reshape(4, 128).T)

    wv = np.zeros((C, 260), dtype=np.float32)
    wvb1 = np.zeros((1, 260), dtype=np.float32)
    for h in range(N_HEADS):
        rows = range(h * 192 + 128, h * 192 + 192)
        wv[:, h * 65:h * 65 + 64] = proj_w[rows, :].T
        wvb1[0, h * 65:h * 65 + 64] = proj_b[rows]
        wvb1[0, h * 65 + 64] = 1.0
    wv = wv.astype(ml_dtypes.bfloat16)
    wvb2 = np.ascontiguousarray(
        np.broadcast_to(np.concatenate([wvb1, wvb1], axis=1), (128, 520))
    )

    wo = np.ascontiguousarray(out_w.T).astype(ml_dtypes.bfloat16)
    ob = np.ascontiguousarray(out_b.reshape(2, 128).T)
    return dict(wqk=wqk, bqk=bqk, wv=wv, wvb2=wvb2, wo=wo, ob=ob)


def kernel(x, proj_w, proj_b, out_w, out_b, _trace=False):
    from concourse.bass_utils import run_bass_kernel_spmd

    x = np.asarray(x, dtype=np.float32)
    proj_w = np.asarray(proj_w, dtype=np.float32)
    proj_b = np.asarray(proj_b, dtype=np.float32)
    out_w = np.asarray(out_w, dtype=np.float32)
    out_b = np.asarray(out_b, dtype=np.float32)

    if "nc" not in _CACHE:
        _CACHE["nc"] = _build()
    nc = _CACHE["nc"]

    w = _prep_weights(proj_w, proj_b, out_w, out_b)
    xs = np.ascontiguousarray(x.reshape(B, C, N))
    xsbf = xs.astype(ml_dtypes.bfloat16)
    in_maps = [
        dict(w, x=np.ascontiguousarray(xs[i * BPC:(i + 1) * BPC]),
             xbf=np.ascontiguousarray(xsbf[i * BPC:(i + 1) * BPC]))
        for i in range(NCORES)
    ]
    res = run_bass_kernel_spmd(nc, in_maps, core_ids=list(range(NCORES)), trace=_trace)
    out = np.concatenate([r["out"] for r in res.results], axis=0)
    out = out.reshape(B, C, H, W)
    if _trace:
        _CACHE["last_result"] = res
    return out


# revision 23
# speedup vs baseline: 1.4066x; 1.0378x over previous
"""AttnBlock Trainium2 Bass kernel.

Data-parallel over batch across 8 NeuronCores (4 batch elements each, full
weights on every core). Everything on-chip is feature-major ([feat, token]),
so the pipeline needs no transposes anywhere.

The kernel is paced by the Scalar engine: softmax exp is 16.8M elements per
core and ACT runs 1 elem/lane/cycle @1.2GHz => ~147us floor (128 ACT ops of
[128, 1024]). The structure keeps every other engine hidden under that:

  x[b]                -> X [C=256, N=1024] (the input's natural layout)
  QK proj (PE bf16)   -> Qst/Kst [128, N], head pair stacked 64+64 rows;
                         bias folded into the DVE psum->sbuf copy
  V proj              -> V [N, 4, 65] [token, dim]; bias + ones column
                         (softmax denominator trick) folded into the copy
  scores              -> sp [128, 2, 512] psum: both heads x one i-half;
                         the head pair is row-tiled (tile_position (0,0) /
                         (64,0)) and issued back-to-back so both heads
                         stream through the PE array concurrently; two sp
                         tiles ping-pong so next scores overlap current exp
  P = exp(scale*s)    -> ONE ACT op per (jc, ic) [128, 1024] psum -> bf16
  ctx = [V|1]^T P     -> psum [65, N] per head; row 64 accumulates Z.
                         ctx matmuls are emitted TWO periods late so the
                         in-order PE queue never parks on the ctx-slot WAR
                         (released by the copy below) while ACT starves.
  normalize           -> ONE DVE copy [65, N] frees the ctx psum slot;
                         Z row is then copied out of SBUF (partition 0),
                         GPSIMD partition_broadcast, DVE recip + multiply
  out proj            -> OUT^T [C, N]; bias + fp32 residual fused in the
                         DVE psum->sbuf pass

PSUM (8 banks, exactly full): sp0, sp1 [128,1024] scores ping-pong, also
time-shared by the projection psums, which are emitted as <=1.1us fillers
one per jc inside the attention loops (the PE slack under the ACT pace);
c0, c1 [65,1024] ctx accumulators, released 1.2us after each pack.

Filler schedule per batch b (one before each jc's scores):
  pack0: V(b+1) pair0/1, Q1(b) half0/1, K1(b) half0/1, outproj(b-1) co0
  pack1: outproj(b-1) co1, V(b+1) pair2/3, Q0(b+1) half0/1, K0(b+1) half0/1
so next-batch Q/K are ready exactly at the batch boundary and the ACT
stream never waits on projections.

Matmul operands are bf16 (converted host-side; fp32 PSUM accumulation).
"""

import numpy as np
import ml_dtypes

N_HEADS = 4
D_K = 64
SCALE = D_K ** (-0.5)
B, C, H, W = 32, 256, 32, 32
N = H * W           # 1024 tokens
NCORES = 8
BPC = B // NCORES   # 4 batch elements per core

_CACHE = {}


def _build():
    import concourse.bacc as bacc
    import concourse.mybir as mybir
    from concourse.tile import TileContext

    dt = mybir.dt
    f32 = dt.float32
    bf16 = dt.bfloat16
    EXP = mybir.ActivationFunctionType.Exp
    ADD = mybir.AluOpType.add
    MULT = mybir.AluOpType.mult

    nc = bacc.Bacc()
    x = nc.dram_tensor("x", [BPC, C, N], f32, kind="ExternalInput")
    xbf = nc.dram_tensor("xbf", [BPC, C, N], bf16, kind="ExternalInput")
    wqk = nc.dram_tensor("wqk", [C, 512], bf16, kind="ExternalInput")
    bqk = nc.dram_tensor("bqk", [128, 4], f32, kind="ExternalInput")
    wv = nc.dram_tensor("wv", [C, 260], bf16, kind="ExternalInput")
    wvb2 = nc.dram_tensor("wvb2", [128, 520], f32, kind="ExternalInput")
    wo = nc.dram_tensor("wo", [C, C], bf16, kind="ExternalInput")
    ob = nc.dram_tensor("ob", [128, 2], f32, kind="ExternalInput")
    out = nc.dram_tensor("out", [BPC, C, N], f32, kind="ExternalOutput")

    with TileContext(nc) as tc:
        with (
            tc.tile_pool(name="consts", bufs=1) as consts,
            tc.tile_pool(name="xp", bufs=4) as xp,
            tc.tile_pool(name="qkp", bufs=4) as qkp,
            tc.tile_pool(name="vp", bufs=2) as vp,
            tc.tile_pool(name="pp", bufs=10) as pp,
            tc.tile_pool(name="miscp", bufs=4) as miscp,
            tc.tile_pool(name="outp", bufs=4) as outp,
            tc.tile_pool(name="psum", bufs=1, space="PSUM") as psum,
        ):
            # ---- load constants once (already bf16 host-side) ----
            wqk_sb = [consts.tile([128, 512], bf16, name=f"wqk{cc}") for cc in range(2)]
            wv_sb = [consts.tile([128, 260], bf16, name=f"wv{cc}") for cc in range(2)]
            wo_sb = [consts.tile([128, 256], bf16, name=f"wo{cc}") for cc in range(2)]
            bqk_sb = consts.tile([128, 4], f32, name="bqk_sb")
            wvb2_sb = consts.tile([128, 520], f32, name="wvb2_sb")
            ob_sb = consts.tile([128, 2], f32, name="ob_sb")
            for cc in range(2):
                nc.sync.dma_start(out=wqk_sb[cc][:], in_=wqk[cc * 128:(cc + 1) * 128, :])
                nc.sync.dma_start(out=wv_sb[cc][:], in_=wv[cc * 128:(cc + 1) * 128, :])
                nc.sync.dma_start(out=wo_sb[cc][:], in_=wo[cc * 128:(cc + 1) * 128, :])
            nc.sync.dma_start(out=bqk_sb[:], in_=bqk[:])
            nc.sync.dma_start(out=wvb2_sb[:], in_=wvb2[:])
            nc.sync.dma_start(out=ob_sb[:], in_=ob[:])
            warmup = consts.tile([1, 4], f32, name="warmup")
            nc.scalar.activation(warmup[:], bqk_sb[0:1, 0:4], EXP)

            xcs, xcrs, qks, vss, osbs, cns = {}, {}, {}, {}, {}, {}

            def emit_x_load(b):
                xc = [xp.tile([128, N], f32, name=f"xc{cc}", tag=f"xc{cc}")
                      for cc in range(2)]
                xcr = [xp.tile([128, N], bf16, name=f"xcr{cc}", tag=f"xcr{cc}")
                       for cc in range(2)]
                # bf16 copies first: they gate the QK projection (critical
                # path at startup); the fp32 residual isn't needed until
                # the out projection
                for cc in range(2):
                    nc.sync.dma_start(out=xcr[cc][:], in_=xbf[b, cc * 128:(cc + 1) * 128, :])
                for cc in range(2):
                    nc.sync.dma_start(out=xc[cc][:], in_=x[b, cc * 128:(cc + 1) * 128, :])
                xcs[b] = xc
                xcrs[b] = xcr

            def emit_qk_half(b, p, qk, fc):
                # one fc-half of one Q/K projection tile; psum borrows the
                # sp slots so it never waits on the normalize chain
                if b not in qks:
                    qks[b] = [[None, None], [None, None]]
                xcr = xcrs[b]
                qkps = psum.tile([128, 512], f32, name="qkps", tag=f"sp{qk}")
                col0 = p * 256 + qk * 128
                fs = slice(fc * 512, (fc + 1) * 512)
                for cc in range(2):
                    nc.tensor.matmul(
                        qkps[:],
                        wqk_sb[cc][:, col0:col0 + 128],
                        xcr[cc][:, fs],
                        start=(cc == 0), stop=(cc == 1),
                    )
                if fc == 0:
                    qks[b][p][qk] = qkp.tile([128, N], bf16, name=f"qk{p}{qk}")
                nc.vector.tensor_scalar(
                    qks[b][p][qk][:, fs], qkps[:],
                    bqk_sb[:, 2 * p + qk:2 * p + qk + 1],
                    None, ADD,
                )

            def emit_v_pair(b, pr):
                # two 128-token V chunks through one psum tile + one DVE op
                if b not in vss:
                    vss[b] = vp.tile([128, 8, 260], bf16, name="v_sb", tag="v")
                xcr = xcrs[b]
                vps = psum.tile([128, 2, 512], f32, name="vps", tag=f"sp{pr % 2}")
                for k in range(2):
                    jt = 2 * pr + k
                    js = slice(jt * 128, (jt + 1) * 128)
                    for cc in range(2):
                        nc.tensor.matmul(
                            vps[:, k, 0:260],
                            xcr[cc][:, js], wv_sb[cc][:],
                            start=(cc == 0), stop=(cc == 1),
                        )
                nc.vector.scalar_tensor_tensor(
                    vss[b][:, 2 * pr:2 * pr + 2, :], vps[:, :, 0:260],
                    1.0, wvb2_sb[:], MULT, ADD,
                )

            def emit_outproj_half(b, co, fc):
                if (b, co) not in osbs:
                    osbs[(b, co)] = outp.tile([128, N], f32, name="osb")
                osb = osbs[(b, co)]
                ctxn = cns[b]
                fs = slice(fc * 512, (fc + 1) * 512)
                ops = psum.tile([128, 512], f32, name="ops", tag=f"sp{co}")
                for kc in range(2):
                    nc.tensor.matmul(
                        ops[:],
                        wo_sb[kc][:, co * 128:(co + 1) * 128],
                        ctxn[kc][:, fs],
                        start=(kc == 0), stop=(kc == 1),
                    )
                nc.vector.scalar_tensor_tensor(
                    osb[:, fs], ops[:], ob_sb[:, co:co + 1], xcs[b][co][:, fs],
                    ADD, ADD,
                )
                if fc == 1:
                    nc.sync.dma_start(
                        out=out[b, co * 128:(co + 1) * 128, :], in_=osb[:]
                    )

            def emit_pack(b, p, fillers):
                qst, kst = qks[b][p][0], qks[b][p][1]
                v_sb = vss[b]
                ctxps = [
                    psum.tile([65, N], f32, name=f"ctx{hl}", tag=f"c{hl}")
                    for hl in range(2)
                ]
                pending = []  # ctx matmuls delayed two periods

                def emit_ctx(jc, ic, pt):
                    isl = slice(ic * 512, (ic + 1) * 512)
                    for hl in range(2):
                        h = 2 * p + hl
                        nc.tensor.matmul(
                            ctxps[hl][:, isl],
                            v_sb[:, jc, h * 65:(h + 1) * 65],
                            pt[:, hl, :],
                            start=(jc == 0), stop=(jc == 7),
                        )

                for jc in range(8):
                    js = slice(jc * 128, (jc + 1) * 128)
                    for ic in range(2):
                        isl = slice(ic * 512, (ic + 1) * 512)
                        sp = psum.tile([128, 2, 512], f32, name="sp",
                                       tag=f"sp{ic}")
                        for hl in range(2):
                            hs = slice(hl * 64, (hl + 1) * 64)
                            nc.tensor.matmul(
                                sp[:, hl, :],
                                kst[hs, js],
                                qst[hs, isl],
                                start=True, stop=True,
                                tile_position=(hl * 64, 0),
                            )
                        pt = pp.tile([128, 2, 512], bf16, name="pt", tag="pt")
                        nc.scalar.activation(pt[:], sp[:], EXP, scale=SCALE)
                        pending.append((jc, ic, pt))
                        if len(pending) > 2:
                            emit_ctx(*pending.pop(0))
                    # fillers go AFTER this jc's work: their sp-slot WAR wait
                    # then coincides with the wait the next scores would have
                    # had anyway, instead of head-of-line blocking the queue
                    if fillers[jc] is not None:
                        fillers[jc]()
                for args in pending:
                    emit_ctx(*args)
                # normalize: single copy frees the ctx psum slot; Z row is
                # re-copied from SBUF down to partition 0 (gpsimd broadcast
                # can only source partition 0), everything else off-path
                cn = miscp.tile([128, N], bf16, name=f"ctxn{p}", tag="cn")
                for hl in range(2):
                    cu = miscp.tile([65, N], f32, name="cu", tag="cu", bufs=3)
                    nc.vector.tensor_copy(cu[:], ctxps[hl][:])
                    z_sb = miscp.tile([1, N], f32, name="z_sb", tag="z", bufs=3)
                    nc.vector.tensor_copy(z_sb[:], cu[64:65, :])
                    zb = miscp.tile([64, N], f32, name="zb", tag="zb", bufs=2)
                    nc.gpsimd.partition_broadcast(zb[:], z_sb[0:1, :])
                    rzb = miscp.tile([64, N], f32, name="rzb", tag="rzb", bufs=2)
                    nc.vector.reciprocal_approx_fast(rzb[:], zb[:])
                    nc.vector.tensor_tensor(
                        cn[hl * 64:(hl + 1) * 64, :],
                        cu[0:64, :],
                        rzb[:],
                        MULT,
                    )
                return cn

            # prologue: batch 0's pack-0 Q/K + all of V, plus x loads
            emit_x_load(0)
            for qk in range(2):
                for fc in range(2):
                    emit_qk_half(0, 0, qk, fc)
            for pr in range(4):
                emit_v_pair(0, pr)
            emit_x_load(1)

            def sched(b):
                """filler lists for pack0 / pack1 of batch b"""
                nxt = b + 1 if b + 1 < BPC else None
                prv = b - 1 if b >= 1 else None
                f0 = [
                    (lambda: emit_v_pair(nxt, 0)) if nxt is not None else None,
                    (lambda: emit_v_pair(nxt, 1)) if nxt is not None else None,
                    (lambda: emit_qk_half(b, 1, 0, 0)),
                    (lambda: emit_qk_half(b, 1, 0, 1)),
                    (lambda: emit_qk_half(b, 1, 1, 0)),
                    (lambda: emit_qk_half(b, 1, 1, 1)),
                    (lambda: emit_outproj_half(prv, 0, 0)) if prv is not None else None,
                    (lambda: emit_outproj_half(prv, 0, 1)) if prv is not None else None,
                ]
                f1 = [
                    (lambda: emit_outproj_half(prv, 1, 0)) if prv is not None else None,
                    (lambda: emit_outproj_half(prv, 1, 1)) if prv is not None else None,
                    (lambda: emit_v_pair(nxt, 2)) if nxt is not None else None,
                    (lambda: emit_v_pair(nxt, 3)) if nxt is not None else None,
                    (lambda: emit_qk_half(nxt, 0, 0, 0)) if nxt is not None else None,
                    (lambda: emit_qk_half(nxt, 0, 0, 1)) if nxt is not None else None,
                    (lambda: emit_qk_half(nxt, 0, 1, 0)) if nxt is not None else None,
                    (lambda: emit_qk_half(nxt, 0, 1, 1)) if nxt is not None else None,
                ]
                return f0, f1

            for b in range(BPC):
                f0, f1 = sched(b)
                cn0 = emit_pack(b, 0, f0)
                cn1 = emit_pack(b, 1, f1)
                cns[b] = [cn0, cn1]
                if b + 2 < BPC:
                    emit_x_load(b + 2)
            # tail: last batch's out projection
            for co in range(2):
                for fc in range(2):
                    emit_outproj_half(BPC - 1, co, fc)

    nc.compile()
    return nc


def _prep_weights(proj_w, proj_b, out_w, out_b):
    qk_cols = []
    for p in range(2):
        for qk in range(2):
            for hl in range(2):
                h = 2 * p + hl
                base = h * 192 + qk * 64
                qk_cols.extend(range(base, base + 64))
    wqk = np.ascontiguousarray(proj_w[qk_cols, :].T).astype(ml_dtypes.bfloat16)
    bqk = np.ascontiguousarray(proj_b[qk_cols].reshape(4, 128).T)

    wv = np.zeros((C, 260), dtype=np.float32)
    wvb1 = np.zeros((1, 260), dtype=np.float32)
    for h in range(N_HEADS):
        rows = range(h * 192 + 128, h * 192 + 192)
        wv[:, h * 65:h * 65 + 64] = proj_w[rows, :].T
        wvb1[0, h * 65:h * 65 + 64] = proj_b[rows]
        wvb1[0, h * 65 + 64] = 1.0
    wv = wv.astype(ml_dtypes.bfloat16)
    wvb2 = np.ascontiguousarray(
        np.broadcast_to(np.concatenate([wvb1, wvb1], axis=1), (128, 520))
    )

    wo = np.ascontiguousarray(out_w.T).astype(ml_dtypes.bfloat16)
    ob = np.ascontiguousarray(out_b.reshape(2, 128).T)
    return dict(wqk=wqk, bqk=bqk, wv=wv, wvb2=wvb2, wo=wo, ob=ob)


def kernel(x, proj_w, proj_b, out_w, out_b, _trace=False):
    from concourse.bass_utils import run_bass_kernel_spmd

    x = np.asarray(x, dtype=np.float32)
    proj_w = np.asarray(proj_w, dtype=np.float32)
    proj_b = np.asarray(proj_b, dtype=np.float32)
    out_w = np.asarray(out_w, dtype=np.float32)
    out_b = np.asarray(out_b, dtype=np.float32)

    if "nc" not in _CACHE:
        _CACHE["nc"] = _build()
    nc = _CACHE["nc"]

    w = _prep_weights(proj_w, proj_b, out_w, out_b)
    xs = np.ascontiguousarray(x.reshape(B, C, N))
    xsbf = xs.astype(ml_dtypes.bfloat16)
    in_maps = [
        dict(w, x=np.ascontiguousarray(xs[i * BPC:(i + 1) * BPC]),
             xbf=np.ascontiguousarray(xsbf[i * BPC:(i + 1) * BPC]))
        for i in range(NCORES)
    ]
    res = run_bass_kernel_spmd(nc, in_maps, core_ids=list(range(NCORES)), trace=_trace)
    out = np.concatenate([r["out"] for r in res.results], axis=0)
    out = out.reshape(B, C, H, W)
    if _trace:
        _CACHE["last_result"] = res
    return out
